# revision 24
# baseline (speedup 1.0000x reference)
"""BTT (block tensor-train) structured FC kernel for Trainium2, 8-core data parallel.

Math: y[b, (oa ob oc od)] = sum_blk sum_{r*} F0[ia,oa,ra] F1[ib,ob,rb] F2[ic,oc,rc]
F3[id,od,rd] C[rd,rc,rb,ra] x[b, (ia ib ic id)]  with all mode dims 8, ranks 2.

Host folds factors into:
  G[icid, blk, q=(rc,rd), ocod] = F2[ic,oc,rc]*F3[id,od,rd]          (stage A rhs)
  W[blk, q, iaib, oaob] = sum_{ra,rb} C[rd,rc,rb,ra] F0[ia,oa,ra] F1[ib,ob,rb]
Sharding is pure batch data-parallel (128 rows per core).

The whole kernel runs in 64x64 PE-array tiling mode -- four independent 64x64
systolic tiles (tile_position=(64r, 64c)) stream concurrently, halving PE time
vs the full 128-wide array for these K=64 matmuls.  That makes the ACT/DVE
PSUM->SBUF copies of the stage-A intermediate the wall, so the design paces
everything by the two copy engines; stage-B matmul bundles are emitted in
FRONT of each stage-A pair so they execute during the pair's PSUM-ring wait.
"""

import numpy as np

N_CORES = 8
B_CORE = 128
NPAIR = 64
NGROUP = 4
PAIRS_PER_GROUP = 16

_CACHE = {}

_XT_SPLIT = [2, 6, 8, 8, 8, 8, 8, 8, 8]

N_WARM = 7
WEAVE_LAG = 4


def _rc(e):
    c = e % 2
    r = (e % 2 + e // 2) % 2
    return r, c


def _fold_weights(cores, factors):
    cores = np.asarray(cores, dtype=np.float64)
    factors = np.asarray(factors, dtype=np.float64)
    G = np.zeros((64, 4, 4, 64), np.float64)
    W = np.zeros((4, 4, 64, 64), np.float64)
    for blk in range(4):
        F0, F1, F2, F3 = (factors[blk, j] for j in range(4))
        C = cores[blk]
        G[:, blk] = np.einsum("cxr,dys->cdrsxy", F2, F3).reshape(64, 4, 64)
        w = np.einsum("srqp,axp,byq->srabxy", C, F0, F1).transpose(1, 0, 2, 3, 4, 5)
        W[blk] = w.reshape(4, 64, 64)
    g2 = G.reshape(64, 1024)
    w3 = W.reshape(16, 64, 64)
    import ml_dtypes
    g_dup = np.concatenate([g2, g2], axis=0)
    w_dup = np.concatenate([w3.transpose(1, 0, 2),
                            w3.transpose(1, 0, 2)], axis=0)
    return (g_dup.astype(ml_dtypes.bfloat16),
            np.ascontiguousarray(w_dup).astype(ml_dtypes.bfloat16))


def _build_nc():
    import concourse.mybir as mybir
    from concourse import bacc
    from concourse.tile import TileContext

    f32 = mybir.dt.float32
    bf16 = mybir.dt.bfloat16

    nc = bacc.Bacc("TRN2", target_bir_lowering=False, debug=False,
                   num_devices=N_CORES)
    xt_d = nc.dram_tensor("xt", [128, NPAIR, 64], bf16, kind="ExternalInput")
    g_d = nc.dram_tensor("g", [128, 1024], bf16, kind="ExternalInput")
    w_d = nc.dram_tensor("w", [128, 16, 64], bf16, kind="ExternalInput")
    y_d = nc.dram_tensor("y", [128, 4096], bf16, kind="ExternalOutput")

    with TileContext(nc) as tc:
        with tc.tile_pool(name="const", bufs=1) as const, \
             tc.tile_pool(name="upool", bufs=4) as upool:

            g_sb = const.tile([128, 1024], bf16, tag="g_sb")
            w_sb = const.tile([128, 16, 64], bf16, tag="w_sb")
            xz_tiles = []
            pair_tile = []
            for j, n_p in enumerate(_XT_SPLIT):
                xzj = const.tile([128, n_p, 64], bf16, tag=f"xz{j}")
                for o in range(n_p):
                    pair_tile.append((j, o))
                xz_tiles.append(xzj)
            off = 0
            for j, n_p in enumerate(_XT_SPLIT):
                q = nc.scalar if j == 2 else nc.sync
                q.dma_start(xz_tiles[j][:], xt_d[:, off:off + n_p, :])
                if j == 0:
                    nc.scalar.dma_start(g_sb[:, 0:512], g_d[:, 0:512])
                elif j == 1:
                    nc.scalar.dma_start(g_sb[:, 512:1024], g_d[:, 512:1024])
                off += n_p
            nc.scalar.dma_start(w_sb[:], w_d[:])
            warm = const.tile([128, 512], bf16, tag="warm")
            nc.vector.memset(warm[:], 0.0)

            ugs = [upool.tile([128, PAIRS_PER_GROUP, 1024], bf16,
                              name=f"ug{g}", tag="ug")
                   for g in range(NGROUP)]
            y_sb = const.tile([128, NGROUP, 1024], bf16, tag="y_sb")

            with tc.tile_pool(name="apsum", bufs=3, space="PSUM") as apsum, \
                 tc.tile_pool(name="bpsum", bufs=1, space="PSUM") as bpsum:
                wps = bpsum.tile([128, 1024], f32, tag="bps")
                # full-array warmups: 64x64-tiled matmuls register less PE
                # activity to the HAM clock-gate monitor, so warm up with
                # 128x128 matmuls (one tiling-mode drain afterwards).
                for i in range(N_WARM):
                    nc.tensor.matmul(wps[:, 0:512],
                                     warm[:, 0:128],
                                     warm[:, 0:512],
                                     start=True, stop=True,
                                     tile_position=(0, 0))

                # greedy balance by estimated engine busy-time: ACT runs
                # (352+FD)/1.2 ns, DVE (120+FD)/0.96 ns per copy
                busy = [0.0, 0.0]

                def emit_copy(dst, src, engine=None):
                    fd = dst.free_size()
                    cs = (352 + fd) / 1.2
                    cv = (120 + fd) / 0.96
                    if engine is None:
                        engine = 0 if busy[0] + cs <= busy[1] + cv else 1
                    if engine == 0:
                        nc.scalar.copy(dst, src)
                        busy[0] += cs
                    else:
                        nc.vector.tensor_copy(dst, src)
                        busy[1] += cv
                    copy_rr[0] += 1
                copy_rr = [0]

                yps_tiles = {}

                def a_pair(ph):
                    g, pl = ph // PAIRS_PER_GROUP, ph % PAIRS_PER_GROUP
                    jt, ot = pair_tile[ph]
                    pp = apsum.tile([128, 1024], f32, name=f"pp{ph}",
                                    tag="aps")
                    for half in range(2):
                        for c in range(2):
                            e = 2 * ph + c
                            r, _ = _rc(e)
                            nc.tensor.matmul(
                                pp[64 * c:64 * c + 64,
                                   512 * half:512 * half + 512],
                                xz_tiles[jt][64 * r:64 * r + 64, ot, :],
                                g_sb[64 * r:64 * r + 64,
                                     512 * half:512 * half + 512],
                                start=True, stop=True,
                                tile_position=(64 * r, 64 * c))
                    ug_flat = ugs[g][:].rearrange("p a b -> p (a b)")
                    if ph >= NPAIR - 3:
                        # end-game: drain the last pairs on both engines at
                        # once so the final stage-B mini-phase starts sooner
                        emit_copy(ug_flat[:, 1024 * pl:1024 * pl + 512],
                                  pp[:, 0:512], engine=0)
                        emit_copy(ug_flat[:, 1024 * pl + 512:1024 * (pl + 1)],
                                  pp[:, 512:1024], engine=1)
                    else:
                        emit_copy(ug_flat[:, 1024 * pl:1024 * (pl + 1)],
                                  pp[:])

                def mk_yps(g):
                    if g not in yps_tiles:
                        yps_tiles[g] = bpsum.tile(
                            [128, 1024], f32, name=f"yps{g}", tag="bps")
                    return yps_tiles[g]

                def b_bundle(g, mu, k, s0, s1, sloc):
                    yps = mk_yps(g)
                    n = (s1 - s0) * 64
                    coff = sloc * 64
                    for ch in range(2):
                        nc.tensor.matmul(
                            yps[64 * mu:64 * mu + 64,
                                512 * ch + coff:512 * ch + coff + n],
                            w_sb[64 * ch:64 * ch + 64, k, :],
                            ugs[g][64 * ch:64 * ch + 64, s0:s1,
                                   64 * k:64 * k + 64],
                            start=(k == 0), stop=(k == 15),
                            tile_position=(64 * ch, 64 * mu))

                def make_bundles():
                    bundles = []
                    for g in range(NGROUP):
                        fine = (g == NGROUP - 1)
                        if not fine:
                            dep = 16 * g + 15
                            for k in range(16):
                                for mu in range(2):
                                    bundles.append((dep, lambda g=g, mu=mu,
                                                    k=k: b_bundle(
                                                        g, mu, k, 8 * mu,
                                                        8 * mu + 8, 0)))
                        else:
                            # two 8-pair mini-phases with mu0/mu1 interleaved
                            # (all 4 PE tiles).  ch outputs stay in DIFFERENT
                            # PSUM banks (512-stride): two concurrent tiles
                            # draining into the same bank wedges the device.
                            def b_fine(g, half, mu, k):
                                yps = mk_yps(g)
                                s0 = 8 * half + 4 * mu
                                for ch in range(2):
                                    nc.tensor.matmul(
                                        yps[64 * mu:64 * mu + 64,
                                            512 * ch + 256 * half:
                                            512 * ch + 256 * half + 256],
                                        w_sb[64 * ch:64 * ch + 64, k, :],
                                        ugs[g][64 * ch:64 * ch + 64,
                                               s0:s0 + 4,
                                               64 * k:64 * k + 64],
                                        start=(k == 0), stop=(k == 15),
                                        tile_position=(64 * ch, 64 * mu))
                            for half in range(2):
                                dep = 16 * g + 8 * half + 7
                                for k in range(16):
                                    for mu in range(2):
                                        bundles.append(
                                            (dep, lambda g=g, half=half,
                                             mu=mu, k=k: b_fine(
                                                 g, half, mu, k)))
                        def wb(g=g, fine=fine):
                            yps = yps_tiles[g]
                            if not fine:
                                dst = y_sb[:, g, :]
                                emit_copy(dst, yps[:])
                                nc.sync.dma_start(
                                    y_d[:, 1024 * g:1024 * (g + 1)], dst)
                            else:
                                ypv = yps[:].rearrange(
                                    "p (c h n) -> p c h n", c=2, h=2)
                                ysv = y_sb[:].rearrange(
                                    "p gg (c h n) -> p gg c h n", c=2, h=2)
                                ydv = y_d[:].rearrange(
                                    "p (gg c h n) -> p gg c h n",
                                    gg=NGROUP, c=2, h=2)
                                nc.scalar.copy(ysv[:, g, :, 0, :],
                                               ypv[:, :, 0, :])
                                nc.sync.dma_start(ydv[:, g, :, 0, :],
                                                  ysv[:, g, :, 0, :])
                                nc.vector.tensor_copy(ysv[:, g, :, 1, :],
                                                      ypv[:, :, 1, :])
                                nc.scalar.dma_start(ydv[:, g, :, 1, :],
                                                    ysv[:, g, :, 1, :])
                        bundles.append(
                            (16 * g + (19 if g < NGROUP - 1 else 15), wb))
                    return bundles

                bundles = make_bundles()

                # First three pairs: lo-half matmuls first (only need g_lo,
                # which lands ~2.5us before g_hi), warm matmuls to bridge the
                # g_hi arrival, then the hi halves and copies.  Keeps the PE
                # dense through the DMA ramp so the HAM clock-gate opens early.
                def a_pair_half(ph, half, pp):
                    g = ph // PAIRS_PER_GROUP
                    pl = ph % PAIRS_PER_GROUP
                    jt, ot = pair_tile[ph]
                    for c in range(2):
                        e = 2 * ph + c
                        r, _ = _rc(e)
                        nc.tensor.matmul(
                            pp[64 * c:64 * c + 64,
                               512 * half:512 * half + 512],
                            xz_tiles[jt][64 * r:64 * r + 64, ot, :],
                            g_sb[64 * r:64 * r + 64,
                                 512 * half:512 * half + 512],
                            start=True, stop=True,
                            tile_position=(64 * r, 64 * c))
                    if half == 1:
                        ug_flat = ugs[g][:].rearrange("p a b -> p (a b)")
                        emit_copy(ug_flat[:, 1024 * pl:1024 * (pl + 1)],
                                  pp[:])

                pps = {}
                for ph in range(3):
                    pps[ph] = apsum.tile([128, 1024], f32, name=f"pp{ph}",
                                         tag="aps")
                    a_pair_half(ph, 0, pps[ph])
                for i in range(3):
                    hw = 64 * (i % 2)
                    nc.tensor.matmul(wps[hw:hw + 64, 0:512],
                                     warm[hw:hw + 64, 0:64],
                                     warm[hw:hw + 64, 0:512],
                                     start=True, stop=True,
                                     tile_position=(hw, hw))
                for ph in range(3):
                    a_pair_half(ph, 1, pps[ph])

                bi = 0
                for ph in range(3, NPAIR):
                    n_emitted = 0
                    while bi < len(bundles) and \
                            bundles[bi][0] <= ph - WEAVE_LAG and \
                            n_emitted < 3:
                        bundles[bi][1]()
                        bi += 1
                        n_emitted += 1
                    a_pair(ph)
                while bi < len(bundles):
                    bundles[bi][1]()
                    bi += 1

    nc.compile()
    return nc


def kernel(inputs, cores, factors, trace=False):
    import ml_dtypes

    x = np.ascontiguousarray(np.asarray(inputs, dtype=np.float32))
    assert x.shape == (N_CORES * B_CORE, 4096), x.shape
    g_dup, w_dup = _fold_weights(cores, factors)

    from concourse.bass_utils import run_bass_kernel_spmd

    if "nc" not in _CACHE:
        _CACHE["nc"] = _build_nc()
    nc = _CACHE["nc"]

    in_maps = []
    for cidx in range(N_CORES):
        xc = x[cidx * B_CORE:(cidx + 1) * B_CORE].reshape(128, 64, 64)
        xt = np.zeros((128, NPAIR, 64), np.float32)
        for e in range(128):
            r, _ = _rc(e)
            xt[64 * r:64 * r + 64, e // 2, :] = xc[e].T
        in_maps.append({"xt": xt.astype(ml_dtypes.bfloat16),
                        "g": g_dup, "w": w_dup})

    res = run_bass_kernel_spmd(nc, in_maps, core_ids=list(range(N_CORES)),
                               trace=trace)
    _CACHE["last_result"] = res

    out = np.empty((N_CORES * B_CORE, 4096), np.float32)
    for cidx in range(N_CORES):
        yp = np.asarray(res.results[cidx]["y"], dtype=np.float32)
        yg = yp.reshape(2, 64, NGROUP, 1024)
        yc = out[cidx * B_CORE:(cidx + 1) * B_CORE].reshape(128, 64, 64)
        for e in range(128):
            _, c = _rc(e)
            ph = e // 2
            g, pl = ph // 16, ph % 16
            if g < NGROUP - 1:
                mu, s8 = pl // 8, pl % 8
                col = 512 * c + 64 * s8
            else:
                half, mu, s4 = pl // 8, (pl % 8) // 4, pl % 4
                col = 512 * c + 256 * half + 64 * s4
            yc[e] = yg[mu, :, g, col:col + 64]
    return out


# revision 25
# speedup vs baseline: 1.2011x; 1.2011x over previous
"""BTT (block tensor-train) structured FC kernel for Trainium2, 8-core data parallel.

Math: y[b, (oa ob oc od)] = sum_blk sum_{r*} F0[ia,oa,ra] F1[ib,ob,rb] F2[ic,oc,rc]
F3[id,od,rd] C[rd,rc,rb,ra] x[b, (ia ib ic id)]  with all mode dims 8, ranks 2.

Host folds factors into:
  G[icid, blk, q=(rc,rd), ocod] = F2[ic,oc,rc]*F3[id,od,rd]          (stage A rhs)
  W[blk, q, iaib, oaob] = sum_{ra,rb} C[rd,rc,rb,ra] F0[ia,oa,ra] F1[ib,ob,rb]
Sharding is pure batch data-parallel (128 rows per core).

The whole kernel runs in 64x64 PE-array tiling mode -- four independent 64x64
systolic tiles (tile_position=(64r, 64c)) stream concurrently, halving PE time
vs the full 128-wide array for these K=64 matmuls.  That makes the ACT/DVE
PSUM->SBUF copies of the stage-A intermediate the wall, so the design paces
everything by the two copy engines; stage-B matmul bundles are emitted in
FRONT of each stage-A pair so they execute during the pair's PSUM-ring wait.
"""

import numpy as np

N_CORES = 8
B_CORE = 128
NPAIR = 64
NGROUP = 4
PAIRS_PER_GROUP = 16

_CACHE = {}

_XT_SPLIT = [2, 6, 8, 8, 8, 8, 8, 8, 8]

N_WARM = 7
WEAVE_LAG = 4


def _rc(e):
    c = e % 2
    r = (e % 2 + e // 2) % 2
    return r, c


def _fold_weights(cores, factors):
    cores = np.asarray(cores, dtype=np.float64)
    factors = np.asarray(factors, dtype=np.float64)
    G = np.zeros((64, 4, 4, 64), np.float64)
    W = np.zeros((4, 4, 64, 64), np.float64)
    for blk in range(4):
        F0, F1, F2, F3 = (factors[blk, j] for j in range(4))
        C = cores[blk]
        G[:, blk] = np.einsum("cxr,dys->cdrsxy", F2, F3).reshape(64, 4, 64)
        w = np.einsum("srqp,axp,byq->srabxy", C, F0, F1).transpose(1, 0, 2, 3, 4, 5)
        W[blk] = w.reshape(4, 64, 64)
    g2 = G.reshape(64, 1024)
    w3 = W.reshape(16, 64, 64)
    import ml_dtypes
    g_dup = np.concatenate([g2, g2], axis=0)
    w_dup = np.concatenate([w3.transpose(1, 0, 2),
                            w3.transpose(1, 0, 2)], axis=0)
    return (g_dup.astype(ml_dtypes.bfloat16),
            np.ascontiguousarray(w_dup).astype(ml_dtypes.bfloat16))


def _build_nc():
    import concourse.mybir as mybir
    from concourse import bacc
    from concourse.tile import TileContext

    f32 = mybir.dt.float32
    bf16 = mybir.dt.bfloat16

    nc = bacc.Bacc("TRN2", target_bir_lowering=False, debug=False,
                   num_devices=N_CORES)
    xt_d = nc.dram_tensor("xt", [128, NPAIR, 64], bf16, kind="ExternalInput")
    g_d = nc.dram_tensor("g", [128, 1024], bf16, kind="ExternalInput")
    w_d = nc.dram_tensor("w", [128, 16, 64], bf16, kind="ExternalInput")
    y_d = nc.dram_tensor("y", [128, 4096], bf16, kind="ExternalOutput")

    with TileContext(nc) as tc:
        with tc.tile_pool(name="const", bufs=1) as const, \
             tc.tile_pool(name="upool", bufs=4) as upool:

            g_sb = const.tile([128, 1024], bf16, tag="g_sb")
            w_sb = const.tile([128, 16, 64], bf16, tag="w_sb")
            xz_tiles = []
            pair_tile = []
            for j, n_p in enumerate(_XT_SPLIT):
                xzj = const.tile([128, n_p, 64], bf16, tag=f"xz{j}")
                for o in range(n_p):
                    pair_tile.append((j, o))
                xz_tiles.append(xzj)
            off = 0
            for j, n_p in enumerate(_XT_SPLIT):
                q = nc.scalar if j == 2 else nc.sync
                q.dma_start(xz_tiles[j][:], xt_d[:, off:off + n_p, :])
                if j == 0:
                    nc.scalar.dma_start(g_sb[:, 0:512], g_d[:, 0:512])
                elif j == 1:
                    nc.scalar.dma_start(g_sb[:, 512:1024], g_d[:, 512:1024])
                off += n_p
            nc.scalar.dma_start(w_sb[:], w_d[:])
            warm = const.tile([128, 512], bf16, tag="warm")
            nc.vector.memset(warm[:], 0.0)

            ugs = [upool.tile([128, PAIRS_PER_GROUP, 1024], bf16,
                              name=f"ug{g}", tag="ug")
                   for g in range(NGROUP)]
            y_sb = const.tile([128, NGROUP, 1024], bf16, tag="y_sb")

            with tc.tile_pool(name="apsum", bufs=3, space="PSUM") as apsum, \
                 tc.tile_pool(name="bpsum", bufs=1, space="PSUM") as bpsum:
                wps = bpsum.tile([128, 1024], f32, tag="bps")
                # full-array warmups: 64x64-tiled matmuls register less PE
                # activity to the HAM clock-gate monitor, so warm up with
                # 128x128 matmuls (one tiling-mode drain afterwards).
                for i in range(N_WARM):
                    nc.tensor.matmul(wps[:, 0:512],
                                     warm[:, 0:128],
                                     warm[:, 0:512],
                                     start=True, stop=True,
                                     tile_position=(0, 0))

                # greedy balance by estimated engine busy-time: ACT runs
                # (352+FD)/1.2 ns, DVE (120+FD)/0.96 ns per copy
                busy = [0.0, 0.0]

                def emit_copy(dst, src, engine=None):
                    fd = dst.free_size()
                    cs = (352 + fd) / 1.2
                    cv = (120 + fd) / 0.96
                    if engine is None:
                        engine = 0 if busy[0] + cs <= busy[1] + cv else 1
                    if engine == 0:
                        nc.scalar.copy(dst, src)
                        busy[0] += cs
                    else:
                        nc.vector.tensor_copy(dst, src)
                        busy[1] += cv
                    copy_rr[0] += 1
                copy_rr = [0]

                yps_tiles = {}

                def a_pair(ph):
                    g, pl = ph // PAIRS_PER_GROUP, ph % PAIRS_PER_GROUP
                    jt, ot = pair_tile[ph]
                    pp = apsum.tile([128, 1024], f32, name=f"pp{ph}",
                                    tag="aps")
                    for half in range(2):
                        for c in range(2):
                            e = 2 * ph + c
                            r, _ = _rc(e)
                            nc.tensor.matmul(
                                pp[64 * c:64 * c + 64,
                                   512 * half:512 * half + 512],
                                xz_tiles[jt][64 * r:64 * r + 64, ot, :],
                                g_sb[64 * r:64 * r + 64,
                                     512 * half:512 * half + 512],
                                start=True, stop=True,
                                tile_position=(64 * r, 64 * c))
                    ug_flat = ugs[g][:].rearrange("p a b -> p (a b)")
                    if ph >= NPAIR - 3:
                        # end-game: drain the last pairs on both engines at
                        # once so the final stage-B mini-phase starts sooner
                        emit_copy(ug_flat[:, 1024 * pl:1024 * pl + 512],
                                  pp[:, 0:512], engine=0)
                        emit_copy(ug_flat[:, 1024 * pl + 512:1024 * (pl + 1)],
                                  pp[:, 512:1024], engine=1)
                    else:
                        emit_copy(ug_flat[:, 1024 * pl:1024 * (pl + 1)],
                                  pp[:])

                def mk_yps(g):
                    if g not in yps_tiles:
                        yps_tiles[g] = bpsum.tile(
                            [128, 1024], f32, name=f"yps{g}", tag="bps")
                    return yps_tiles[g]

                def b_bundle(g, mu, k, s0, s1, sloc):
                    yps = mk_yps(g)
                    n = (s1 - s0) * 64
                    coff = sloc * 64
                    for ch in range(2):
                        nc.tensor.matmul(
                            yps[64 * mu:64 * mu + 64,
                                512 * ch + coff:512 * ch + coff + n],
                            w_sb[64 * ch:64 * ch + 64, k, :],
                            ugs[g][64 * ch:64 * ch + 64, s0:s1,
                                   64 * k:64 * k + 64],
                            start=(k == 0), stop=(k == 15),
                            tile_position=(64 * ch, 64 * mu))

                def make_bundles():
                    bundles = []
                    for g in range(NGROUP):
                        fine = (g == NGROUP - 1)
                        if not fine:
                            dep = 16 * g + 15
                            for k in range(16):
                                for mu in range(2):
                                    bundles.append((dep, lambda g=g, mu=mu,
                                                    k=k: b_bundle(
                                                        g, mu, k, 8 * mu,
                                                        8 * mu + 8, 0)))
                        else:
                            # two 8-pair mini-phases with mu0/mu1 interleaved
                            # (all 4 PE tiles).  ch outputs stay in DIFFERENT
                            # PSUM banks (512-stride): two concurrent tiles
                            # draining into the same bank wedges the device.
                            def b_fine(g, half, mu, k):
                                yps = mk_yps(g)
                                s0 = 8 * half + 4 * mu
                                for ch in range(2):
                                    nc.tensor.matmul(
                                        yps[64 * mu:64 * mu + 64,
                                            512 * ch + 256 * half:
                                            512 * ch + 256 * half + 256],
                                        w_sb[64 * ch:64 * ch + 64, k, :],
                                        ugs[g][64 * ch:64 * ch + 64,
                                               s0:s0 + 4,
                                               64 * k:64 * k + 64],
                                        start=(k == 0), stop=(k == 15),
                                        tile_position=(64 * ch, 64 * mu))
                            for half in range(2):
                                dep = 16 * g + 8 * half + 7
                                for k in range(16):
                                    for mu in range(2):
                                        bundles.append(
                                            (dep, lambda g=g, half=half,
                                             mu=mu, k=k: b_fine(
                                                 g, half, mu, k)))
                        def wb(g=g, fine=fine):
                            yps = yps_tiles[g]
                            if not fine:
                                dst = y_sb[:, g, :]
                                emit_copy(dst, yps[:])
                                nc.sync.dma_start(
                                    y_d[:, 1024 * g:1024 * (g + 1)], dst)
                            else:
                                nc.scalar.copy(y_sb[:, g, 0:512],
                                               yps[:, 0:512])
                                nc.vector.tensor_copy(y_sb[:, g, 512:1024],
                                                      yps[:, 512:1024])
                                nc.sync.dma_start(
                                    y_d[:, 1024 * g:1024 * g + 512],
                                    y_sb[:, g, 0:512])
                                nc.scalar.dma_start(
                                    y_d[:, 1024 * g + 512:1024 * (g + 1)],
                                    y_sb[:, g, 512:1024])
                        bundles.append((16 * g + 15, wb))
                    return bundles

                bundles = make_bundles()

                # First three pairs: lo-half matmuls first (only need g_lo,
                # which lands ~2.5us before g_hi), warm matmuls to bridge the
                # g_hi arrival, then the hi halves and copies.  Keeps the PE
                # dense through the DMA ramp so the HAM clock-gate opens early.
                def a_pair_half(ph, half, pp):
                    g = ph // PAIRS_PER_GROUP
                    pl = ph % PAIRS_PER_GROUP
                    jt, ot = pair_tile[ph]
                    for c in range(2):
                        e = 2 * ph + c
                        r, _ = _rc(e)
                        nc.tensor.matmul(
                            pp[64 * c:64 * c + 64,
                               512 * half:512 * half + 512],
                            xz_tiles[jt][64 * r:64 * r + 64, ot, :],
                            g_sb[64 * r:64 * r + 64,
                                 512 * half:512 * half + 512],
                            start=True, stop=True,
                            tile_position=(64 * r, 64 * c))
                    if half == 1:
                        ug_flat = ugs[g][:].rearrange("p a b -> p (a b)")
                        emit_copy(ug_flat[:, 1024 * pl:1024 * (pl + 1)],
                                  pp[:])

                pps = {}
                for ph in range(3):
                    pps[ph] = apsum.tile([128, 1024], f32, name=f"pp{ph}",
                                         tag="aps")
                    a_pair_half(ph, 0, pps[ph])
                for i in range(3):
                    hw = 64 * (i % 2)
                    nc.tensor.matmul(wps[hw:hw + 64, 0:512],
                                     warm[hw:hw + 64, 0:64],
                                     warm[hw:hw + 64, 0:512],
                                     start=True, stop=True,
                                     tile_position=(hw, hw))
                for ph in range(3):
                    a_pair_half(ph, 1, pps[ph])

                bi = 0
                for ph in range(3, NPAIR):
                    n_emitted = 0
                    while bi < len(bundles) and \
                            bundles[bi][0] <= ph - WEAVE_LAG and \
                            n_emitted < 3:
                        bundles[bi][1]()
                        bi += 1
                        n_emitted += 1
                    a_pair(ph)
                while bi < len(bundles):
                    bundles[bi][1]()
                    bi += 1

    nc.compile()
    return nc


def kernel(inputs, cores, factors, trace=False):
    import ml_dtypes

    x = np.ascontiguousarray(np.asarray(inputs, dtype=np.float32))
    assert x.shape == (N_CORES * B_CORE, 4096), x.shape
    g_dup, w_dup = _fold_weights(cores, factors)

    from concourse.bass_utils import run_bass_kernel_spmd

    if "nc" not in _CACHE:
        _CACHE["nc"] = _build_nc()
    nc = _CACHE["nc"]

    in_maps = []
    for cidx in range(N_CORES):
        xc = x[cidx * B_CORE:(cidx + 1) * B_CORE].reshape(128, 64, 64)
        xt = np.zeros((128, NPAIR, 64), np.float32)
        for e in range(128):
            r, _ = _rc(e)
            xt[64 * r:64 * r + 64, e // 2, :] = xc[e].T
        in_maps.append({"xt": xt.astype(ml_dtypes.bfloat16),
                        "g": g_dup, "w": w_dup})

    res = run_bass_kernel_spmd(nc, in_maps, core_ids=list(range(N_CORES)),
                               trace=trace)
    _CACHE["last_result"] = res

    out = np.empty((N_CORES * B_CORE, 4096), np.float32)
    for cidx in range(N_CORES):
        yp = np.asarray(res.results[cidx]["y"], dtype=np.float32)
        yg = yp.reshape(2, 64, NGROUP, 1024)
        yc = out[cidx * B_CORE:(cidx + 1) * B_CORE].reshape(128, 64, 64)
        for e in range(128):
            _, c = _rc(e)
            ph = e // 2
            g, pl = ph // 16, ph % 16
            if g < NGROUP - 1:
                mu, s8 = pl // 8, pl % 8
                col = 512 * c + 64 * s8
            else:
                half, mu, s4 = pl // 8, (pl % 8) // 4, pl % 4
                col = 512 * c + 256 * half + 64 * s4
            yc[e] = yg[mu, :, g, col:col + 64]
    return out
